# revision 27
# baseline (speedup 1.0000x reference)
"""Maxish pooling kernel for Trainium2 (8 NeuronCores, data-parallel).

Reference math (per row of length N):
    m  = max(x)
    pos = m * sum(exp(2*rt)) / sum(exp(rt)),  rt = (x-m)/m   (s == 1)
    out = m > 0 ? pos : (m < 0 ? m : 0)

Bias-free refactor: with v = exp(x/m) = e * exp(rt),
    pos = m * sum(v^2) / (e * sum(v))
so the per-tile affine is a single per-row scale (no bias) and the 1/e
constant folds into the final fixup.

Layout (block assignment): partition p owns rows [p*T, (p+1)*T) of the
per-core slab, so each chunk load is ONE contiguous 16 KiB descriptor
per partition (vs 16 x 1 KiB interleaved) and the final store is a
single fully-contiguous [128, T] DMA - no PE transpose needed. x is
cast to bf16 during the SWDGE load; rel-err budget (2e-2) dwarfs it.

Engine split per chunk of G tiles (fast path, s == 1):
  - DVE: pairwise bf16 max folds (2x mode) + 1x reduce; per-PAIR 3D
    bn_stats [p,2,n] -> both sum(v) and sum(v^2) (halves bn fixed
    cost); bn->S1/S2 fixup ONCE at the end on [128,T] strided views.
  - ACT: one shared big exp over the z-route tiles + per-tile exps for
    `plain` tiles; `as_tiles` tiles use exp/square accum_out sums.
  - GpSimd: SWDGE desc-gen + z_gps single-scalar affines.

Pipelining: loads issued `look_ahead` chunks early; plain exps emitted
before the z-dependent big exp (ACT in-order); bn for chunk c deferred
until after chunk c+1's work is queued so DVE never head-blocks on
fresh ACT output.
"""

import numpy as np

P = 128
N = 256
SMALL = 1e-8
INV_E = float(np.exp(-1.0))

def _build(n_rows: int, s: float, G: int = 16, fold_k: int = 3,
           z_dve: int = 0, z_gps: int = 12, as_tiles: int = 2,
           x_bufs: int = 4, u_bufs: int = 3, f_bufs: int = 3,
           xdt: str = "bf16", exact_recip: bool = False,
           clamp: bool = False, look_ahead: int = 2,
           pair_bn: bool = True, bn_lag: int = 2, exp_splits: int = 2):
    from concourse import bacc, mybir
    from concourse.tile import TileContext

    f32 = mybir.dt.float32
    bf16 = mybir.dt.bfloat16
    dt_x = bf16 if xdt == "bf16" else f32
    Act = mybir.ActivationFunctionType
    Alu = mybir.AluOpType
    Ax = mybir.AxisListType

    assert n_rows % (P * G) == 0
    T = n_rows // P          # tiles of [128, N]; also out elems/partition
    C = T // G               # chunks of G tiles
    fast = (s == 1.0)
    assert 0 <= fold_k <= 4 and N % (1 << fold_k) == 0
    assert 0 <= z_dve + z_gps + as_tiles <= G
    if fast and pair_bn:
        # pair-interleaved bn needs an even split everywhere
        assert (z_dve + z_gps) % 2 == 0 and (G - as_tiles) % 2 == 0

    bn_lag = max(1, bn_lag)
    u_bufs = max(u_bufs, bn_lag + 2)

    nc = bacc.Bacc("TRN2", target_bir_lowering=False, debug=False,
                   num_devices=8)
    x_d = nc.declare_dram_parameter("x", [n_rows * N], f32, isOutput=False)
    out_d = nc.declare_dram_parameter("out", [n_rows], f32, isOutput=True)
    # partition p owns rows [p*T, (p+1)*T): flat view [128, T*N]
    x_pv = x_d[:].rearrange("(p q) -> p q", p=P)
    out_pv = out_d[:].rearrange("(p t) -> p t", p=P)

    def recip(dst, src):
        if exact_recip:
            nc.vector.reciprocal(dst, src)
        else:
            nc.vector.reciprocal_approx_fast(dst, src)

    with TileContext(nc) as tc:
        with (
            tc.tile_pool(name="xp", bufs=x_bufs) as xp,
            tc.tile_pool(name="up", bufs=u_bufs) as up,
            tc.tile_pool(name="fold", bufs=f_bufs) as fp,
            tc.tile_pool(name="stat", bufs=1) as statp,
            tc.tile_pool(name="consts", bufs=6) as cpool,
        ):
            M = statp.tile([P, T], f32, tag="M")       # per-row max
            S1 = statp.tile([P, T], f32, tag="S1")     # sum v^2
            S2 = statp.tile([P, T], f32, tag="S2")     # sum v
            R = statp.tile([P, T], f32, tag="R")       # result / scratch
            T2 = statp.tile([P, T], f32, tag="T2")     # post scratch
            T3 = statp.tile([P, T], f32, tag="T3")     # post scratch
            T4 = statp.tile([P, T], f32, tag="T4")     # post scratch
            MK = statp.tile([P, T], mybir.dt.uint8, tag="MK")  # m>0 mask
            nbst = (T // 2) if pair_bn else T
            BST = statp.tile([P, nbst * 6], f32, tag="BST")  # bn tuples
            if as_tiles:
                S1A = statp.tile([P, C * as_tiles], f32, tag="S1A")
                S2A = statp.tile([P, C * as_tiles], f32, tag="S2A")

            gb_ = G - as_tiles

            def emit_bn(put, pc):
                if pair_bn:
                    for q in range(gb_ // 2):
                        j = pc * (G // 2) + q
                        nc.vector.bn_stats(
                            out=BST[:, j * 6:(j + 1) * 6],
                            in_=put[:, q * 2 * N:(q + 1) * 2 * N])
                else:
                    for g in range(gb_):
                        j = pc * G + g
                        nc.vector.bn_stats(
                            out=BST[:, j * 6:(j + 1) * 6],
                            in_=put[:, g * N:(g + 1) * N])

            # pre-issue loads so SWDGE desc-gen (in-order Pool queue)
            # isn't stuck behind z ops that wait on mid-chunk results
            xts: list = [None] * C

            def load(ci):
                xt = xp.tile([P, G * N], dt_x, tag="x")
                src = x_pv[:, ci * G * N:(ci + 1) * G * N]
                xdma = nc.gpsimd if dt_x == bf16 else nc.sync
                xdma.dma_start(out=xt[:], in_=src)
                xts[ci] = xt

            for ci in range(min(look_ahead, C)):
                load(ci)

            pendings: list = []  # (ut, chunk) awaiting deferred bn

            for c in range(C):
                if c + look_ahead < C:
                    load(c + look_ahead)
                xt = xts[c]
                xts[c] = None

                # DVE pairwise max folds (2x on bf16), then 1x reduce
                red_in = xt[:].rearrange("p (g n) -> p g n", n=N)
                w = N
                for k in range(fold_k):
                    w //= 2
                    ft = fp.tile([P, G * w], dt_x, tag=f"f{k}")
                    f3 = ft[:].rearrange("p (g n) -> p g n", n=w)
                    nc.vector.tensor_tensor(
                        out=f3, in0=red_in[:, :, 0:w],
                        in1=red_in[:, :, w:2 * w], op=Alu.max)
                    red_in = f3
                mg = M[:, c * G:(c + 1) * G]
                nc.vector.tensor_reduce(out=mg, in_=red_in, axis=Ax.X,
                                        op=Alu.max)

                # rg = 1/(m+eps) (clamped >= 0 so m<0 rows don't
                # overflow exp). For max-of-256 randn inputs m <= 0 has
                # probability 2^-256 and any overflow stays in its own
                # row (masked by the m>0 select), so the eps add and
                # the clamp are skipped unless `clamp` is set.
                cb = cpool.tile([P, 4 * G], f32, tag="cb")
                rg = cb[:, 0:G]
                if clamp:
                    nc.vector.tensor_scalar_add(rg, mg, SMALL)
                    recip(rg, rg)
                    nc.vector.tensor_scalar_max(rg, rg, 0.0)
                else:
                    recip(rg, mg)

                if fast:
                    zc = z_dve + z_gps
                    gb = G - as_tiles        # tiles covered by bn
                    ut = up.tile([P, G * N], dt_x, tag="u")

                    # pair-interleaved view: tile g of pair q=g//2 lands
                    # on element lanes (2n + g%2) of the pair block, so
                    # ONE bn_stats per pair separates the two rows into
                    # its even/odd stat halves.
                    def uo(g):
                        if not pair_bn:
                            return ut[:, g * N:(g + 1) * N]
                        uv = ut[:].rearrange("p (q n two) -> p q two n",
                                             two=2, n=N)
                        return uv[:, g // 2, g % 2]

                    # plain tiles first so the in-order ACT queue can
                    # run them while GPS/DVE compute the z affines
                    for g in range(zc, gb):
                        nc.scalar.activation(
                            out=uo(g), in_=xt[:, g * N:(g + 1) * N],
                            func=Act.Exp, scale=rg[:, g:g + 1])
                    # ACT-sum tiles: sums via accum (exp then square)
                    for g in range(gb, G):
                        fs = slice(g * N, (g + 1) * N)
                        j = c * as_tiles + (g - gb)
                        nc.scalar.activation(
                            out=ut[:, fs], in_=xt[:, fs], func=Act.Exp,
                            scale=rg[:, g:g + 1],
                            accum_out=S2A[:, j:j + 1])
                        nc.scalar.activation(
                            out=ut[:, fs], in_=ut[:, fs], func=Act.Square,
                            accum_out=S1A[:, j:j + 1])
                    # z affines write interleaved z; the big exp then
                    # runs elementwise in-place on the flat region.
                    # ONE broadcast tensor_tensor per engine covers its
                    # whole z range: in1 = rg with a stride-0 inner dim
                    # (per-tile scalar, amortizes the per-op overhead).
                    def z_affine(eng, g0, g1):
                        if g1 <= g0:
                            return
                        if pair_bn:
                            assert g0 % 2 == 0 and g1 % 2 == 0
                            qs = slice(g0 // 2, g1 // 2)
                            ov = ut[:].rearrange(
                                "p (q n two) -> p q two n", two=2,
                                n=N)[:, qs]
                            iv = xt[:].rearrange(
                                "p (q two n) -> p q two n", two=2,
                                n=N)[:, qs]
                            rv = rg[:, g0:g1].rearrange(
                                "p (q two) -> p q two", two=2
                            ).unsqueeze(-1).broadcast_to(
                                [P, (g1 - g0) // 2, 2, N])
                        else:
                            ov = ut[:].rearrange(
                                "p (g n) -> p g n", n=N)[:, g0:g1]
                            iv = xt[:].rearrange(
                                "p (g n) -> p g n", n=N)[:, g0:g1]
                            rv = rg[:, g0:g1].unsqueeze(-1).broadcast_to(
                                [P, g1 - g0, N])
                        eng.tensor_tensor(out=ov, in0=iv, in1=rv,
                                          op=Alu.mult)

                    z_affine(nc.vector, 0, z_dve)
                    z_affine(nc.gpsimd, z_dve, zc)
                    # in-place big exp over the z region, split so ACT
                    # can start before ALL affines are done (the affine
                    # batch is the longest per-chunk stage)
                    if zc:
                        ns = max(1, min(exp_splits, zc // 2))
                        # pair-aligned split boundaries
                        npair = zc // 2
                        bounds = [2 * ((npair * i) // ns) for i in range(ns)]
                        bounds.append(zc)
                        for i in range(ns):
                            zs = slice(bounds[i] * N, bounds[i + 1] * N)
                            nc.scalar.activation(out=ut[:, zs],
                                                 in_=ut[:, zs],
                                                 func=Act.Exp)

                    # deferred bn, lagged bn_lag chunks so the
                    # cross-engine recip->affine->exp->bn chain never
                    # cycles back into the DVE queue (convoy breaker)
                    pendings.append((ut, c))
                    if len(pendings) > bn_lag:
                        put, pc = pendings.pop(0)
                        emit_bn(put, pc)
                else:
                    bg = cb[:, G:2 * G]
                    c1 = cb[:, 2 * G:3 * G]
                    b1 = cb[:, 3 * G:4 * G]
                    # general s: u = exp(s*(x*rg - 1)) etc., two passes
                    nc.vector.tensor_scalar_mul(bg, rg, s)
                    nc.vector.tensor_scalar_mul(c1, rg, 1.0 + s)
                    ut = up.tile([P, G * N], dt_x, tag="u")
                    for g in range(G):
                        fs = slice(g * N, (g + 1) * N)
                        j = c * G + g
                        nc.scalar.activation(
                            out=ut[:, fs], in_=xt[:, fs], func=Act.Exp,
                            scale=bg[:, g:g + 1], bias=float(-s),
                            accum_out=S2[:, j:j + 1])
                        nc.scalar.activation(
                            out=ut[:, fs], in_=xt[:, fs], func=Act.Exp,
                            scale=c1[:, g:g + 1], bias=float(-(1.0 + s)),
                            accum_out=S1[:, j:j + 1])

            if fast:
                for put, pc in pendings:
                    emit_bn(put, pc)
                pendings = []
                gb = gb_

                if pair_bn:
                    # bn fixup: pair (c,q) covers tiles (2q, 2q+1) of
                    # chunk c; 6-tuple: [n_e, mu_e, M2_e, n_o, mu_o,
                    # M2_o] where the even half IS tile 2q and the odd
                    # half tile 2q+1 (the interleaved write). Per row:
                    # S2 = N*mu, S1 = M2 + N*mu^2. Views slice q < qb
                    # so the unwritten as-pair slots are never read.
                    Q, qb = G // 2, gb // 2
                    bs = BST[:].rearrange("p (c q s) -> p s c q",
                                          s=6, q=Q)[:, :, :, 0:qb]
                    s1v = S1[:].rearrange("p (c q two) -> p two c q",
                                          two=2, q=Q)[:, :, :, 0:qb]
                    s2v = S2[:].rearrange("p (c q two) -> p two c q",
                                          two=2, q=Q)[:, :, :, 0:qb]
                    fn = float(N)
                    Th = C * qb
                    te3 = T3[:, 0:Th].rearrange("p (c q) -> p c q", q=qb)
                    to3 = T4[:, 0:Th].rearrange("p (c q) -> p c q", q=qb)
                    nc.vector.tensor_scalar_mul(s2v[:, 0], bs[:, 1], fn)
                    nc.vector.tensor_scalar_mul(s2v[:, 1], bs[:, 4], fn)
                    nc.vector.tensor_tensor(te3, bs[:, 1], bs[:, 1],
                                            op=Alu.mult)
                    nc.vector.scalar_tensor_tensor(
                        out=s1v[:, 0], in0=te3, scalar=fn, in1=bs[:, 2],
                        op0=Alu.mult, op1=Alu.add)
                    nc.vector.tensor_tensor(to3, bs[:, 4], bs[:, 4],
                                            op=Alu.mult)
                    nc.vector.scalar_tensor_tensor(
                        out=s1v[:, 1], in0=to3, scalar=fn, in1=bs[:, 5],
                        op0=Alu.mult, op1=Alu.add)
                else:
                    # per-tile bn: merge even/odd halves.
                    # S2 = 128*(mu_e+mu_o);
                    # S1 = M2_e+M2_o+128*(mu_e^2+mu_o^2)
                    # (as-tile columns hold garbage; overwritten below)
                    Q, qb = G, gb
                    bs = BST[:].rearrange("p (c q s) -> p s c q",
                                          s=6, q=Q)[:, :, :, 0:qb]
                    s1v = S1[:].rearrange("p (c q) -> p c q", q=Q)[
                        :, :, 0:qb]
                    s2v = S2[:].rearrange("p (c q) -> p c q", q=Q)[
                        :, :, 0:qb]
                    half = float(N // 2)
                    Th = C * qb
                    t1 = T3[:, 0:Th].rearrange("p (c q) -> p c q", q=qb)
                    t2 = T4[:, 0:Th].rearrange("p (c q) -> p c q", q=qb)
                    t3 = R[:, 0:Th].rearrange("p (c q) -> p c q", q=qb)
                    nc.vector.tensor_tensor(t1, bs[:, 1], bs[:, 4],
                                            op=Alu.add)
                    nc.vector.tensor_scalar_mul(s2v, t1, half)
                    nc.vector.tensor_tensor(t2, bs[:, 1], bs[:, 1],
                                            op=Alu.mult)
                    nc.vector.tensor_tensor(t3, bs[:, 4], bs[:, 4],
                                            op=Alu.mult)
                    nc.vector.tensor_tensor(t2, t2, t3, op=Alu.add)
                    nc.vector.tensor_tensor(t3, bs[:, 2], bs[:, 5],
                                            op=Alu.add)
                    nc.vector.scalar_tensor_tensor(
                        out=s1v, in0=t2, scalar=half, in1=t3,
                        op0=Alu.mult, op1=Alu.add)
                # overwrite the as-tile columns with their ACT-accum
                # sums (single strided copies)
                if as_tiles:
                    s1v = S1[:].rearrange("p (c g) -> p c g", g=G)[
                        :, :, G - as_tiles:G]
                    s2v = S2[:].rearrange("p (c g) -> p c g", g=G)[
                        :, :, G - as_tiles:G]
                    a3 = S1A[:].rearrange("p (c a) -> p c a", a=as_tiles)
                    b3 = S2A[:].rearrange("p (c a) -> p c a", a=as_tiles)
                    nc.vector.tensor_copy(s1v, a3)
                    nc.vector.tensor_copy(s2v, b3)

            # pos = m * S1 / (e * S2) ; out = m>0 ? pos : (m<0 ? m : 0)
            # (1/e factor only in the fast path; general-s path already
            #  bakes the biases into the exps)
            recip(S2[:], S2[:])
            if fast:
                nc.vector.scalar_tensor_tensor(
                    out=S1[:], in0=S1[:], scalar=INV_E, in1=S2[:],
                    op0=Alu.mult, op1=Alu.mult)
            else:
                nc.vector.tensor_tensor(S1[:], S1[:], S2[:], op=Alu.mult)
            nc.vector.tensor_tensor(S1[:], S1[:], M[:], op=Alu.mult)
            # mask of m > 0 (uint8 - CopyPredicated needs an int mask)
            nc.vector.tensor_scalar(MK[:], M[:], 0.0, None, op0=Alu.is_gt)
            nc.vector.tensor_copy(R[:], M[:])
            nc.vector.copy_predicated(out=R[:], mask=MK[:], data=S1[:])

            # block layout -> the store is fully contiguous per partition
            nc.sync.dma_start(out=out_pv, in_=R[:])

    nc.compile()
    return nc


def _run(x: np.ndarray, scale: np.ndarray, trace: bool = False,
         build_kw: dict | None = None, **kw):
    from concourse.bass_utils import run_bass_kernel_spmd

    n_cores = 8
    B, Tm, X, Nn = x.shape          # 32, 256, 64, 256
    assert Nn == N
    rows = B * Tm * X
    rows_per_core = rows // n_cores
    s = float(np.asarray(scale))

    nc = _build(rows_per_core, s, **(build_kw or {}))
    xs = np.ascontiguousarray(np.asarray(x, dtype=np.float32)).reshape(
        n_cores, rows_per_core * N)
    in_maps = [{"x": xs[i]} for i in range(n_cores)]
    res = run_bass_kernel_spmd(nc, in_maps, list(range(n_cores)),
                               trace=trace, **kw)
    # partition p of core i holds rows [p*T, (p+1)*T) -> flat concat
    out = np.concatenate([r["out"].reshape(-1) for r in res.results], axis=0)
    return out.reshape(B, Tm, X).astype(np.float32), res


def kernel(x: np.ndarray, scale: np.ndarray) -> np.ndarray:
    return _run(x, scale)[0]


# revision 29
# speedup vs baseline: 1.0565x; 1.0565x over previous
"""Maxish pooling kernel for Trainium2 (8 NeuronCores, data-parallel).

Reference math (per row of length N):
    m  = max(x)
    pos = m * sum(exp(2*rt)) / sum(exp(rt)),  rt = (x-m)/m   (s == 1)
    out = m > 0 ? pos : (m < 0 ? m : 0)

Bias-free refactor: with v = exp(x/m) = e * exp(rt),
    pos = m * sum(v^2) / (e * sum(v))
so the per-tile affine is a single per-row scale (no bias) and the 1/e
constant folds into the final fixup.

Layout (block assignment): partition p owns rows [p*T, (p+1)*T) of the
per-core slab, so each chunk load is ONE contiguous 16 KiB descriptor
per partition (vs 16 x 1 KiB interleaved) and the final store is a
single fully-contiguous [128, T] DMA - no PE transpose needed. x is
cast to bf16 during the SWDGE load; rel-err budget (2e-2) dwarfs it.

Engine split per chunk of G tiles (fast path, s == 1):
  - DVE: pairwise bf16 max folds (2x mode) + 1x reduce; per-PAIR 3D
    bn_stats [p,2,n] -> both sum(v) and sum(v^2) (halves bn fixed
    cost); bn->S1/S2 fixup ONCE at the end on [128,T] strided views.
  - ACT: one shared big exp over the z-route tiles + per-tile exps for
    `plain` tiles; `as_tiles` tiles use exp/square accum_out sums.
  - GpSimd: SWDGE desc-gen + z_gps single-scalar affines.

Pipelining: loads issued `look_ahead` chunks early; plain exps emitted
before the z-dependent big exp (ACT in-order); bn for chunk c deferred
until after chunk c+1's work is queued so DVE never head-blocks on
fresh ACT output.
"""

import numpy as np

P = 128
N = 256
SMALL = 1e-8
INV_E = float(np.exp(-1.0))

def _build(n_rows: int, s: float, G: int = 16, fold_k: int = 3,
           z_dve: int = 0, z_gps: int = 12, as_tiles: int = 2,
           x_bufs: int = 4, u_bufs: int = 3, f_bufs: int = 3,
           xdt: str = "bf16", exact_recip: bool = False,
           clamp: bool = False, look_ahead: int = 2,
           pair_bn: bool = True, bn_lag: int = 2, exp_splits: int = 2,
           z_group: int = 2):
    from concourse import bacc, mybir
    from concourse.tile import TileContext

    f32 = mybir.dt.float32
    bf16 = mybir.dt.bfloat16
    dt_x = bf16 if xdt == "bf16" else f32
    Act = mybir.ActivationFunctionType
    Alu = mybir.AluOpType
    Ax = mybir.AxisListType

    assert n_rows % (P * G) == 0
    T = n_rows // P          # tiles of [128, N]; also out elems/partition
    C = T // G               # chunks of G tiles
    fast = (s == 1.0)
    assert 0 <= fold_k <= 4 and N % (1 << fold_k) == 0
    assert 0 <= z_dve + z_gps + as_tiles <= G
    if fast and pair_bn:
        # pair-interleaved bn needs an even split everywhere
        assert (z_dve + z_gps) % 2 == 0 and (G - as_tiles) % 2 == 0

    bn_lag = max(1, bn_lag)
    u_bufs = max(u_bufs, bn_lag + 2)

    nc = bacc.Bacc("TRN2", target_bir_lowering=False, debug=False,
                   num_devices=8)
    x_d = nc.declare_dram_parameter("x", [n_rows * N], f32, isOutput=False)
    out_d = nc.declare_dram_parameter("out", [n_rows], f32, isOutput=True)
    # partition p owns rows [p*T, (p+1)*T): flat view [128, T*N]
    x_pv = x_d[:].rearrange("(p q) -> p q", p=P)
    out_pv = out_d[:].rearrange("(p t) -> p t", p=P)

    def recip(dst, src):
        if exact_recip:
            nc.vector.reciprocal(dst, src)
        else:
            nc.vector.reciprocal_approx_fast(dst, src)

    with TileContext(nc) as tc:
        with (
            tc.tile_pool(name="xp", bufs=x_bufs) as xp,
            tc.tile_pool(name="up", bufs=u_bufs) as up,
            tc.tile_pool(name="fold", bufs=f_bufs) as fp,
            tc.tile_pool(name="stat", bufs=1) as statp,
            tc.tile_pool(name="consts", bufs=6) as cpool,
        ):
            M = statp.tile([P, T], f32, tag="M")       # per-row max
            S1 = statp.tile([P, T], f32, tag="S1")     # sum v^2
            S2 = statp.tile([P, T], f32, tag="S2")     # sum v
            R = statp.tile([P, T], f32, tag="R")       # result / scratch
            T2 = statp.tile([P, T], f32, tag="T2")     # post scratch
            T3 = statp.tile([P, T], f32, tag="T3")     # post scratch
            T4 = statp.tile([P, T], f32, tag="T4")     # post scratch
            MK = statp.tile([P, T], mybir.dt.uint8, tag="MK")  # m>0 mask
            nbst = (T // 2) if pair_bn else T
            BST = statp.tile([P, nbst * 6], f32, tag="BST")  # bn tuples
            if as_tiles:
                S1A = statp.tile([P, C * as_tiles], f32, tag="S1A")
                S2A = statp.tile([P, C * as_tiles], f32, tag="S2A")

            gb_ = G - as_tiles

            def emit_bn(put, pc):
                if pair_bn:
                    for q in range(gb_ // 2):
                        j = pc * (G // 2) + q
                        nc.vector.bn_stats(
                            out=BST[:, j * 6:(j + 1) * 6],
                            in_=put[:, q * 2 * N:(q + 1) * 2 * N])
                else:
                    for g in range(gb_):
                        j = pc * G + g
                        nc.vector.bn_stats(
                            out=BST[:, j * 6:(j + 1) * 6],
                            in_=put[:, g * N:(g + 1) * N])

            # pre-issue loads so SWDGE desc-gen (in-order Pool queue)
            # isn't stuck behind z ops that wait on mid-chunk results
            xts: list = [None] * C

            def load(ci):
                xt = xp.tile([P, G * N], dt_x, tag="x")
                src = x_pv[:, ci * G * N:(ci + 1) * G * N]
                xdma = nc.gpsimd if dt_x == bf16 else nc.sync
                xdma.dma_start(out=xt[:], in_=src)
                xts[ci] = xt

            for ci in range(min(look_ahead, C)):
                load(ci)

            pendings: list = []  # (ut, chunk) awaiting deferred bn

            for c in range(C):
                if c + look_ahead < C:
                    load(c + look_ahead)
                xt = xts[c]
                xts[c] = None

                # DVE pairwise max folds (2x on bf16), then 1x reduce
                red_in = xt[:].rearrange("p (g n) -> p g n", n=N)
                w = N
                for k in range(fold_k):
                    w //= 2
                    ft = fp.tile([P, G * w], dt_x, tag=f"f{k}")
                    f3 = ft[:].rearrange("p (g n) -> p g n", n=w)
                    nc.vector.tensor_tensor(
                        out=f3, in0=red_in[:, :, 0:w],
                        in1=red_in[:, :, w:2 * w], op=Alu.max)
                    red_in = f3
                mg = M[:, c * G:(c + 1) * G]
                nc.vector.tensor_reduce(out=mg, in_=red_in, axis=Ax.X,
                                        op=Alu.max)

                # rg = 1/(m+eps) (clamped >= 0 so m<0 rows don't
                # overflow exp). For max-of-256 randn inputs m <= 0 has
                # probability 2^-256 and any overflow stays in its own
                # row (masked by the m>0 select), so the eps add and
                # the clamp are skipped unless `clamp` is set.
                cb = cpool.tile([P, 4 * G], f32, tag="cb")
                rg = cb[:, 0:G]
                if clamp:
                    nc.vector.tensor_scalar_add(rg, mg, SMALL)
                    recip(rg, rg)
                    nc.vector.tensor_scalar_max(rg, rg, 0.0)
                else:
                    recip(rg, mg)

                if fast:
                    zc = z_dve + z_gps
                    gb = G - as_tiles        # tiles covered by bn
                    ut = up.tile([P, G * N], dt_x, tag="u")

                    # pair-interleaved view: tile g of pair q=g//2 lands
                    # on element lanes (2n + g%2) of the pair block, so
                    # ONE bn_stats per pair separates the two rows into
                    # its even/odd stat halves.
                    def uo(g):
                        if not pair_bn:
                            return ut[:, g * N:(g + 1) * N]
                        uv = ut[:].rearrange("p (q n two) -> p q two n",
                                             two=2, n=N)
                        return uv[:, g // 2, g % 2]

                    # plain tiles first so the in-order ACT queue can
                    # run them while GPS/DVE compute the z affines
                    for g in range(zc, gb):
                        nc.scalar.activation(
                            out=uo(g), in_=xt[:, g * N:(g + 1) * N],
                            func=Act.Exp, scale=rg[:, g:g + 1])
                    # ACT-sum tiles: sums via accum (exp then square)
                    for g in range(gb, G):
                        fs = slice(g * N, (g + 1) * N)
                        j = c * as_tiles + (g - gb)
                        nc.scalar.activation(
                            out=ut[:, fs], in_=xt[:, fs], func=Act.Exp,
                            scale=rg[:, g:g + 1],
                            accum_out=S2A[:, j:j + 1])
                        nc.scalar.activation(
                            out=ut[:, fs], in_=ut[:, fs], func=Act.Square,
                            accum_out=S1A[:, j:j + 1])
                    # z affines write interleaved z; the big exp then
                    # runs elementwise in-place on the flat region.
                    # ONE broadcast tensor_tensor per engine covers its
                    # whole z range: in1 = rg with a stride-0 inner dim
                    # (per-tile scalar, amortizes the per-op overhead).
                    def z_affine(eng, g0, g1):
                        if g1 <= g0:
                            return
                        if pair_bn:
                            assert g0 % 2 == 0 and g1 % 2 == 0
                            qs = slice(g0 // 2, g1 // 2)
                            ov = ut[:].rearrange(
                                "p (q n two) -> p q two n", two=2,
                                n=N)[:, qs]
                            iv = xt[:].rearrange(
                                "p (q two n) -> p q two n", two=2,
                                n=N)[:, qs]
                            rv = rg[:, g0:g1].rearrange(
                                "p (q two) -> p q two", two=2
                            ).unsqueeze(-1).broadcast_to(
                                [P, (g1 - g0) // 2, 2, N])
                        else:
                            ov = ut[:].rearrange(
                                "p (g n) -> p g n", n=N)[:, g0:g1]
                            iv = xt[:].rearrange(
                                "p (g n) -> p g n", n=N)[:, g0:g1]
                            rv = rg[:, g0:g1].unsqueeze(-1).broadcast_to(
                                [P, g1 - g0, N])
                        eng.tensor_tensor(out=ov, in0=iv, in1=rv,
                                          op=Alu.mult)

                    # grouped emission: long single ops starve DVE's
                    # 2-port perf mode; ~2-tile bursts don't
                    zg = max(2, 2 * z_group // 2)
                    for g0 in range(0, z_dve, zg):
                        z_affine(nc.vector, g0, min(g0 + zg, z_dve))
                    for g0 in range(z_dve, zc, zg):
                        z_affine(nc.gpsimd, g0, min(g0 + zg, zc))
                    # in-place big exp over the z region, split so ACT
                    # can start before ALL affines are done (the affine
                    # batch is the longest per-chunk stage)
                    if zc:
                        ns = max(1, min(exp_splits, zc // 2))
                        # pair-aligned split boundaries
                        npair = zc // 2
                        bounds = [2 * ((npair * i) // ns) for i in range(ns)]
                        bounds.append(zc)
                        for i in range(ns):
                            zs = slice(bounds[i] * N, bounds[i + 1] * N)
                            nc.scalar.activation(out=ut[:, zs],
                                                 in_=ut[:, zs],
                                                 func=Act.Exp)

                    # deferred bn, lagged bn_lag chunks so the
                    # cross-engine recip->affine->exp->bn chain never
                    # cycles back into the DVE queue (convoy breaker)
                    pendings.append((ut, c))
                    if len(pendings) > bn_lag:
                        put, pc = pendings.pop(0)
                        emit_bn(put, pc)
                else:
                    bg = cb[:, G:2 * G]
                    c1 = cb[:, 2 * G:3 * G]
                    b1 = cb[:, 3 * G:4 * G]
                    # general s: u = exp(s*(x*rg - 1)) etc., two passes
                    nc.vector.tensor_scalar_mul(bg, rg, s)
                    nc.vector.tensor_scalar_mul(c1, rg, 1.0 + s)
                    ut = up.tile([P, G * N], dt_x, tag="u")
                    for g in range(G):
                        fs = slice(g * N, (g + 1) * N)
                        j = c * G + g
                        nc.scalar.activation(
                            out=ut[:, fs], in_=xt[:, fs], func=Act.Exp,
                            scale=bg[:, g:g + 1], bias=float(-s),
                            accum_out=S2[:, j:j + 1])
                        nc.scalar.activation(
                            out=ut[:, fs], in_=xt[:, fs], func=Act.Exp,
                            scale=c1[:, g:g + 1], bias=float(-(1.0 + s)),
                            accum_out=S1[:, j:j + 1])

            if fast:
                for put, pc in pendings:
                    emit_bn(put, pc)
                pendings = []
                gb = gb_

                if pair_bn:
                    # bn fixup: pair (c,q) covers tiles (2q, 2q+1) of
                    # chunk c; 6-tuple: [n_e, mu_e, M2_e, n_o, mu_o,
                    # M2_o] where the even half IS tile 2q and the odd
                    # half tile 2q+1 (the interleaved write). Per row:
                    # S2 = N*mu, S1 = M2 + N*mu^2. Views slice q < qb
                    # so the unwritten as-pair slots are never read.
                    Q, qb = G // 2, gb // 2
                    bs = BST[:].rearrange("p (c q s) -> p s c q",
                                          s=6, q=Q)[:, :, :, 0:qb]
                    s1v = S1[:].rearrange("p (c q two) -> p two c q",
                                          two=2, q=Q)[:, :, :, 0:qb]
                    s2v = S2[:].rearrange("p (c q two) -> p two c q",
                                          two=2, q=Q)[:, :, :, 0:qb]
                    fn = float(N)
                    Th = C * qb
                    te3 = T3[:, 0:Th].rearrange("p (c q) -> p c q", q=qb)
                    to3 = T4[:, 0:Th].rearrange("p (c q) -> p c q", q=qb)
                    nc.vector.tensor_scalar_mul(s2v[:, 0], bs[:, 1], fn)
                    nc.vector.tensor_scalar_mul(s2v[:, 1], bs[:, 4], fn)
                    nc.vector.tensor_tensor(te3, bs[:, 1], bs[:, 1],
                                            op=Alu.mult)
                    nc.vector.scalar_tensor_tensor(
                        out=s1v[:, 0], in0=te3, scalar=fn, in1=bs[:, 2],
                        op0=Alu.mult, op1=Alu.add)
                    nc.vector.tensor_tensor(to3, bs[:, 4], bs[:, 4],
                                            op=Alu.mult)
                    nc.vector.scalar_tensor_tensor(
                        out=s1v[:, 1], in0=to3, scalar=fn, in1=bs[:, 5],
                        op0=Alu.mult, op1=Alu.add)
                else:
                    # per-tile bn: merge even/odd halves.
                    # S2 = 128*(mu_e+mu_o);
                    # S1 = M2_e+M2_o+128*(mu_e^2+mu_o^2)
                    # (as-tile columns hold garbage; overwritten below)
                    Q, qb = G, gb
                    bs = BST[:].rearrange("p (c q s) -> p s c q",
                                          s=6, q=Q)[:, :, :, 0:qb]
                    s1v = S1[:].rearrange("p (c q) -> p c q", q=Q)[
                        :, :, 0:qb]
                    s2v = S2[:].rearrange("p (c q) -> p c q", q=Q)[
                        :, :, 0:qb]
                    half = float(N // 2)
                    Th = C * qb
                    t1 = T3[:, 0:Th].rearrange("p (c q) -> p c q", q=qb)
                    t2 = T4[:, 0:Th].rearrange("p (c q) -> p c q", q=qb)
                    t3 = R[:, 0:Th].rearrange("p (c q) -> p c q", q=qb)
                    nc.vector.tensor_tensor(t1, bs[:, 1], bs[:, 4],
                                            op=Alu.add)
                    nc.vector.tensor_scalar_mul(s2v, t1, half)
                    nc.vector.tensor_tensor(t2, bs[:, 1], bs[:, 1],
                                            op=Alu.mult)
                    nc.vector.tensor_tensor(t3, bs[:, 4], bs[:, 4],
                                            op=Alu.mult)
                    nc.vector.tensor_tensor(t2, t2, t3, op=Alu.add)
                    nc.vector.tensor_tensor(t3, bs[:, 2], bs[:, 5],
                                            op=Alu.add)
                    nc.vector.scalar_tensor_tensor(
                        out=s1v, in0=t2, scalar=half, in1=t3,
                        op0=Alu.mult, op1=Alu.add)
                # overwrite the as-tile columns with their ACT-accum
                # sums (single strided copies)
                if as_tiles:
                    s1v = S1[:].rearrange("p (c g) -> p c g", g=G)[
                        :, :, G - as_tiles:G]
                    s2v = S2[:].rearrange("p (c g) -> p c g", g=G)[
                        :, :, G - as_tiles:G]
                    a3 = S1A[:].rearrange("p (c a) -> p c a", a=as_tiles)
                    b3 = S2A[:].rearrange("p (c a) -> p c a", a=as_tiles)
                    nc.vector.tensor_copy(s1v, a3)
                    nc.vector.tensor_copy(s2v, b3)

            # pos = m * S1 / (e * S2) ; out = m>0 ? pos : (m<0 ? m : 0)
            # (1/e factor only in the fast path; general-s path already
            #  bakes the biases into the exps)
            recip(S2[:], S2[:])
            if fast:
                nc.vector.scalar_tensor_tensor(
                    out=S1[:], in0=S1[:], scalar=INV_E, in1=S2[:],
                    op0=Alu.mult, op1=Alu.mult)
            else:
                nc.vector.tensor_tensor(S1[:], S1[:], S2[:], op=Alu.mult)
            nc.vector.tensor_tensor(S1[:], S1[:], M[:], op=Alu.mult)
            # mask of m > 0 (uint8 - CopyPredicated needs an int mask)
            nc.vector.tensor_scalar(MK[:], M[:], 0.0, None, op0=Alu.is_gt)
            nc.vector.tensor_copy(R[:], M[:])
            nc.vector.copy_predicated(out=R[:], mask=MK[:], data=S1[:])

            # block layout -> the store is fully contiguous per partition
            nc.sync.dma_start(out=out_pv, in_=R[:])

    nc.compile()
    return nc


def _run(x: np.ndarray, scale: np.ndarray, trace: bool = False,
         build_kw: dict | None = None, **kw):
    from concourse.bass_utils import run_bass_kernel_spmd

    n_cores = 8
    B, Tm, X, Nn = x.shape          # 32, 256, 64, 256
    assert Nn == N
    rows = B * Tm * X
    rows_per_core = rows // n_cores
    s = float(np.asarray(scale))

    nc = _build(rows_per_core, s, **(build_kw or {}))
    xs = np.ascontiguousarray(np.asarray(x, dtype=np.float32)).reshape(
        n_cores, rows_per_core * N)
    in_maps = [{"x": xs[i]} for i in range(n_cores)]
    res = run_bass_kernel_spmd(nc, in_maps, list(range(n_cores)),
                               trace=trace, **kw)
    # partition p of core i holds rows [p*T, (p+1)*T) -> flat concat
    out = np.concatenate([r["out"].reshape(-1) for r in res.results], axis=0)
    return out.reshape(B, Tm, X).astype(np.float32), res


def kernel(x: np.ndarray, scale: np.ndarray) -> np.ndarray:
    return _run(x, scale)[0]


# revision 31
# speedup vs baseline: 1.2299x; 1.1642x over previous
"""Maxish pooling kernel for Trainium2 (8 NeuronCores, data-parallel).

Reference math (per row of length N):
    m  = max(x)
    pos = m * sum(exp(2*rt)) / sum(exp(rt)),  rt = (x-m)/m   (s == 1)
    out = m > 0 ? pos : (m < 0 ? m : 0)

Bias-free refactor: with v = exp(x/m) = e * exp(rt),
    pos = m * sum(v^2) / (e * sum(v))
so the per-tile affine is a single per-row scale (no bias) and the 1/e
constant folds into the final fixup.

Layout (block assignment): partition p owns rows [p*T, (p+1)*T) of the
per-core slab, so each chunk load is ONE contiguous 16 KiB descriptor
per partition (vs 16 x 1 KiB interleaved) and the final store is a
single fully-contiguous [128, T] DMA - no PE transpose needed. x is
cast to bf16 during the SWDGE load; rel-err budget (2e-2) dwarfs it.

Engine split per chunk of G tiles (fast path, s == 1):
  - DVE: pairwise bf16 max folds (2x mode) + 1x reduce; per-PAIR 3D
    bn_stats [p,2,n] -> both sum(v) and sum(v^2) (halves bn fixed
    cost); bn->S1/S2 fixup ONCE at the end on [128,T] strided views.
  - ACT: one shared big exp over the z-route tiles + per-tile exps for
    `plain` tiles; `as_tiles` tiles use exp/square accum_out sums.
  - GpSimd: SWDGE desc-gen + z_gps single-scalar affines.

Pipelining: loads issued `look_ahead` chunks early; plain exps emitted
before the z-dependent big exp (ACT in-order); bn for chunk c deferred
until after chunk c+1's work is queued so DVE never head-blocks on
fresh ACT output.
"""

import numpy as np

P = 128
N = 256
SMALL = 1e-8
INV_E = float(np.exp(-1.0))

def _build(n_rows: int, s: float, G: int = 16, fold_k: int = 3,
           z_dve: int = 2, z_gps: int = 8, as_tiles: int = 2,
           x_bufs: int = 4, u_bufs: int = 3, f_bufs: int = 3,
           xdt: str = "bf16", exact_recip: bool = False,
           clamp: bool = False, look_ahead: int = 2,
           pair_bn: bool = True, bn_lag: int = 2, exp_splits: int = 1):
    from concourse import bacc, mybir
    from concourse.tile import TileContext

    f32 = mybir.dt.float32
    bf16 = mybir.dt.bfloat16
    dt_x = bf16 if xdt == "bf16" else f32
    Act = mybir.ActivationFunctionType
    Alu = mybir.AluOpType
    Ax = mybir.AxisListType

    assert n_rows % (P * G) == 0
    T = n_rows // P          # tiles of [128, N]; also out elems/partition
    C = T // G               # chunks of G tiles
    fast = (s == 1.0)
    assert 0 <= fold_k <= 4 and N % (1 << fold_k) == 0
    assert 0 <= z_dve + z_gps + as_tiles <= G
    if fast and pair_bn:
        # pair-interleaved bn needs an even split everywhere
        assert (z_dve + z_gps) % 2 == 0 and (G - as_tiles) % 2 == 0

    bn_lag = max(1, bn_lag)
    u_bufs = max(u_bufs, bn_lag + 2)

    nc = bacc.Bacc("TRN2", target_bir_lowering=False, debug=False,
                   num_devices=8)
    x_d = nc.declare_dram_parameter("x", [n_rows * N], f32, isOutput=False)
    out_d = nc.declare_dram_parameter("out", [n_rows], f32, isOutput=True)
    # partition p owns rows [p*T, (p+1)*T): flat view [128, T*N]
    x_pv = x_d[:].rearrange("(p q) -> p q", p=P)
    out_pv = out_d[:].rearrange("(p t) -> p t", p=P)

    def recip(dst, src):
        if exact_recip:
            nc.vector.reciprocal(dst, src)
        else:
            nc.vector.reciprocal_approx_fast(dst, src)

    with TileContext(nc) as tc:
        with (
            tc.tile_pool(name="xp", bufs=x_bufs) as xp,
            tc.tile_pool(name="up", bufs=u_bufs) as up,
            tc.tile_pool(name="fold", bufs=f_bufs) as fp,
            tc.tile_pool(name="stat", bufs=1) as statp,
            tc.tile_pool(name="consts", bufs=6) as cpool,
        ):
            M = statp.tile([P, T], f32, tag="M")       # per-row max
            S1 = statp.tile([P, T], f32, tag="S1")     # sum v^2
            S2 = statp.tile([P, T], f32, tag="S2")     # sum v
            R = statp.tile([P, T], f32, tag="R")       # result / scratch
            T2 = statp.tile([P, T], f32, tag="T2")     # post scratch
            T3 = statp.tile([P, T], f32, tag="T3")     # post scratch
            T4 = statp.tile([P, T], f32, tag="T4")     # post scratch
            MK = statp.tile([P, T], mybir.dt.uint8, tag="MK")  # m>0 mask
            nbst = (T // 2) if pair_bn else T
            BST = statp.tile([P, nbst * 6], f32, tag="BST")  # bn tuples
            # zero bias vector: the dual-scalar (mult,add) GpSimd
            # tensor_scalar path is ~5x faster than the single-scalar one
            ZB = statp.tile([P, 1], f32, tag="ZB")
            nc.vector.memset(ZB[:], 0.0)
            if as_tiles:
                S1A = statp.tile([P, C * as_tiles], f32, tag="S1A")
                S2A = statp.tile([P, C * as_tiles], f32, tag="S2A")

            gb_ = G - as_tiles

            def emit_bn(put, pc):
                if pair_bn:
                    for q in range(gb_ // 2):
                        j = pc * (G // 2) + q
                        nc.vector.bn_stats(
                            out=BST[:, j * 6:(j + 1) * 6],
                            in_=put[:, q * 2 * N:(q + 1) * 2 * N])
                else:
                    for g in range(gb_):
                        j = pc * G + g
                        nc.vector.bn_stats(
                            out=BST[:, j * 6:(j + 1) * 6],
                            in_=put[:, g * N:(g + 1) * N])

            # pre-issue loads so SWDGE desc-gen (in-order Pool queue)
            # isn't stuck behind z ops that wait on mid-chunk results
            xts: list = [None] * C

            def load(ci):
                xt = xp.tile([P, G * N], dt_x, tag="x")
                src = x_pv[:, ci * G * N:(ci + 1) * G * N]
                xdma = nc.gpsimd if dt_x == bf16 else nc.sync
                xdma.dma_start(out=xt[:], in_=src)
                xts[ci] = xt

            for ci in range(min(look_ahead, C)):
                load(ci)

            pendings: list = []  # (ut, chunk) awaiting deferred bn

            for c in range(C):
                if c + look_ahead < C:
                    load(c + look_ahead)
                xt = xts[c]
                xts[c] = None

                # DVE pairwise max folds (2x on bf16), then 1x reduce
                red_in = xt[:].rearrange("p (g n) -> p g n", n=N)
                w = N
                for k in range(fold_k):
                    w //= 2
                    ft = fp.tile([P, G * w], dt_x, tag=f"f{k}")
                    f3 = ft[:].rearrange("p (g n) -> p g n", n=w)
                    nc.vector.tensor_tensor(
                        out=f3, in0=red_in[:, :, 0:w],
                        in1=red_in[:, :, w:2 * w], op=Alu.max)
                    red_in = f3
                mg = M[:, c * G:(c + 1) * G]
                nc.vector.tensor_reduce(out=mg, in_=red_in, axis=Ax.X,
                                        op=Alu.max)

                # rg = 1/(m+eps) (clamped >= 0 so m<0 rows don't
                # overflow exp). For max-of-256 randn inputs m <= 0 has
                # probability 2^-256 and any overflow stays in its own
                # row (masked by the m>0 select), so the eps add and
                # the clamp are skipped unless `clamp` is set.
                cb = cpool.tile([P, 4 * G], f32, tag="cb")
                rg = cb[:, 0:G]
                if clamp:
                    nc.vector.tensor_scalar_add(rg, mg, SMALL)
                    recip(rg, rg)
                    nc.vector.tensor_scalar_max(rg, rg, 0.0)
                else:
                    recip(rg, mg)

                if fast:
                    zc = z_dve + z_gps
                    gb = G - as_tiles        # tiles covered by bn
                    ut = up.tile([P, G * N], dt_x, tag="u")

                    # pair-interleaved view: tile g of pair q=g//2 lands
                    # on element lanes (2n + g%2) of the pair block, so
                    # ONE bn_stats per pair separates the two rows into
                    # its even/odd stat halves.
                    def uo(g):
                        if not pair_bn:
                            return ut[:, g * N:(g + 1) * N]
                        uv = ut[:].rearrange("p (q n two) -> p q two n",
                                             two=2, n=N)
                        return uv[:, g // 2, g % 2]

                    # plain tiles first so the in-order ACT queue can
                    # run them while GPS/DVE compute the z affines
                    for g in range(zc, gb):
                        nc.scalar.activation(
                            out=uo(g), in_=xt[:, g * N:(g + 1) * N],
                            func=Act.Exp, scale=rg[:, g:g + 1])
                    # ACT-sum tiles: sums via accum (exp then square)
                    for g in range(gb, G):
                        fs = slice(g * N, (g + 1) * N)
                        j = c * as_tiles + (g - gb)
                        nc.scalar.activation(
                            out=ut[:, fs], in_=xt[:, fs], func=Act.Exp,
                            scale=rg[:, g:g + 1],
                            accum_out=S2A[:, j:j + 1])
                        nc.scalar.activation(
                            out=ut[:, fs], in_=ut[:, fs], func=Act.Square,
                            accum_out=S1A[:, j:j + 1])
                    # z affines write interleaved z; the big exp then
                    # runs elementwise in-place on the flat region.
                    # Per-tile dual-scalar tensor_scalar: the GPS
                    # single-scalar variant is ~5x slower, and batched
                    # broadcast tensor_tensor starves DVE's 2-port perf
                    # mode (measured: folds 847 -> 1678+ ns).
                    for g in range(zc):
                        zeng = nc.vector if g < z_dve else nc.gpsimd
                        zeng.tensor_scalar(
                            out=uo(g), in0=xt[:, g * N:(g + 1) * N],
                            scalar1=rg[:, g:g + 1], scalar2=ZB[:, 0:1],
                            op0=Alu.mult, op1=Alu.add)
                    # in-place big exp over the z region, split so ACT
                    # can start before ALL affines are done (the affine
                    # batch is the longest per-chunk stage)
                    if zc:
                        ns = max(1, min(exp_splits, zc // 2))
                        # pair-aligned split boundaries
                        npair = zc // 2
                        bounds = [2 * ((npair * i) // ns) for i in range(ns)]
                        bounds.append(zc)
                        for i in range(ns):
                            zs = slice(bounds[i] * N, bounds[i + 1] * N)
                            nc.scalar.activation(out=ut[:, zs],
                                                 in_=ut[:, zs],
                                                 func=Act.Exp)

                    # deferred bn, lagged bn_lag chunks so the
                    # cross-engine recip->affine->exp->bn chain never
                    # cycles back into the DVE queue (convoy breaker)
                    pendings.append((ut, c))
                    if len(pendings) > bn_lag:
                        put, pc = pendings.pop(0)
                        emit_bn(put, pc)
                else:
                    bg = cb[:, G:2 * G]
                    c1 = cb[:, 2 * G:3 * G]
                    b1 = cb[:, 3 * G:4 * G]
                    # general s: u = exp(s*(x*rg - 1)) etc., two passes
                    nc.vector.tensor_scalar_mul(bg, rg, s)
                    nc.vector.tensor_scalar_mul(c1, rg, 1.0 + s)
                    ut = up.tile([P, G * N], dt_x, tag="u")
                    for g in range(G):
                        fs = slice(g * N, (g + 1) * N)
                        j = c * G + g
                        nc.scalar.activation(
                            out=ut[:, fs], in_=xt[:, fs], func=Act.Exp,
                            scale=bg[:, g:g + 1], bias=float(-s),
                            accum_out=S2[:, j:j + 1])
                        nc.scalar.activation(
                            out=ut[:, fs], in_=xt[:, fs], func=Act.Exp,
                            scale=c1[:, g:g + 1], bias=float(-(1.0 + s)),
                            accum_out=S1[:, j:j + 1])

            if fast:
                for put, pc in pendings:
                    emit_bn(put, pc)
                pendings = []
                gb = gb_

                if pair_bn:
                    # bn fixup: pair (c,q) covers tiles (2q, 2q+1) of
                    # chunk c; 6-tuple: [n_e, mu_e, M2_e, n_o, mu_o,
                    # M2_o] where the even half IS tile 2q and the odd
                    # half tile 2q+1 (the interleaved write). Per row:
                    # S2 = N*mu, S1 = M2 + N*mu^2. Views slice q < qb
                    # so the unwritten as-pair slots are never read.
                    Q, qb = G // 2, gb // 2
                    bs = BST[:].rearrange("p (c q s) -> p s c q",
                                          s=6, q=Q)[:, :, :, 0:qb]
                    s1v = S1[:].rearrange("p (c q two) -> p two c q",
                                          two=2, q=Q)[:, :, :, 0:qb]
                    s2v = S2[:].rearrange("p (c q two) -> p two c q",
                                          two=2, q=Q)[:, :, :, 0:qb]
                    fn = float(N)
                    Th = C * qb
                    te3 = T3[:, 0:Th].rearrange("p (c q) -> p c q", q=qb)
                    to3 = T4[:, 0:Th].rearrange("p (c q) -> p c q", q=qb)
                    nc.vector.tensor_scalar_mul(s2v[:, 0], bs[:, 1], fn)
                    nc.vector.tensor_scalar_mul(s2v[:, 1], bs[:, 4], fn)
                    nc.vector.tensor_tensor(te3, bs[:, 1], bs[:, 1],
                                            op=Alu.mult)
                    nc.vector.scalar_tensor_tensor(
                        out=s1v[:, 0], in0=te3, scalar=fn, in1=bs[:, 2],
                        op0=Alu.mult, op1=Alu.add)
                    nc.vector.tensor_tensor(to3, bs[:, 4], bs[:, 4],
                                            op=Alu.mult)
                    nc.vector.scalar_tensor_tensor(
                        out=s1v[:, 1], in0=to3, scalar=fn, in1=bs[:, 5],
                        op0=Alu.mult, op1=Alu.add)
                else:
                    # per-tile bn: merge even/odd halves.
                    # S2 = 128*(mu_e+mu_o);
                    # S1 = M2_e+M2_o+128*(mu_e^2+mu_o^2)
                    # (as-tile columns hold garbage; overwritten below)
                    Q, qb = G, gb
                    bs = BST[:].rearrange("p (c q s) -> p s c q",
                                          s=6, q=Q)[:, :, :, 0:qb]
                    s1v = S1[:].rearrange("p (c q) -> p c q", q=Q)[
                        :, :, 0:qb]
                    s2v = S2[:].rearrange("p (c q) -> p c q", q=Q)[
                        :, :, 0:qb]
                    half = float(N // 2)
                    Th = C * qb
                    t1 = T3[:, 0:Th].rearrange("p (c q) -> p c q", q=qb)
                    t2 = T4[:, 0:Th].rearrange("p (c q) -> p c q", q=qb)
                    t3 = R[:, 0:Th].rearrange("p (c q) -> p c q", q=qb)
                    nc.vector.tensor_tensor(t1, bs[:, 1], bs[:, 4],
                                            op=Alu.add)
                    nc.vector.tensor_scalar_mul(s2v, t1, half)
                    nc.vector.tensor_tensor(t2, bs[:, 1], bs[:, 1],
                                            op=Alu.mult)
                    nc.vector.tensor_tensor(t3, bs[:, 4], bs[:, 4],
                                            op=Alu.mult)
                    nc.vector.tensor_tensor(t2, t2, t3, op=Alu.add)
                    nc.vector.tensor_tensor(t3, bs[:, 2], bs[:, 5],
                                            op=Alu.add)
                    nc.vector.scalar_tensor_tensor(
                        out=s1v, in0=t2, scalar=half, in1=t3,
                        op0=Alu.mult, op1=Alu.add)
                # overwrite the as-tile columns with their ACT-accum
                # sums (single strided copies)
                if as_tiles:
                    s1v = S1[:].rearrange("p (c g) -> p c g", g=G)[
                        :, :, G - as_tiles:G]
                    s2v = S2[:].rearrange("p (c g) -> p c g", g=G)[
                        :, :, G - as_tiles:G]
                    a3 = S1A[:].rearrange("p (c a) -> p c a", a=as_tiles)
                    b3 = S2A[:].rearrange("p (c a) -> p c a", a=as_tiles)
                    nc.vector.tensor_copy(s1v, a3)
                    nc.vector.tensor_copy(s2v, b3)

            # pos = m * S1 / (e * S2) ; out = m>0 ? pos : (m<0 ? m : 0)
            # (1/e factor only in the fast path; general-s path already
            #  bakes the biases into the exps)
            recip(S2[:], S2[:])
            if fast:
                nc.vector.scalar_tensor_tensor(
                    out=S1[:], in0=S1[:], scalar=INV_E, in1=S2[:],
                    op0=Alu.mult, op1=Alu.mult)
            else:
                nc.vector.tensor_tensor(S1[:], S1[:], S2[:], op=Alu.mult)
            nc.vector.tensor_tensor(S1[:], S1[:], M[:], op=Alu.mult)
            # mask of m > 0 (uint8 - CopyPredicated needs an int mask)
            nc.vector.tensor_scalar(MK[:], M[:], 0.0, None, op0=Alu.is_gt)
            nc.vector.tensor_copy(R[:], M[:])
            nc.vector.copy_predicated(out=R[:], mask=MK[:], data=S1[:])

            # block layout -> the store is fully contiguous per partition
            nc.sync.dma_start(out=out_pv, in_=R[:])

    nc.compile()
    return nc


def _run(x: np.ndarray, scale: np.ndarray, trace: bool = False,
         build_kw: dict | None = None, **kw):
    from concourse.bass_utils import run_bass_kernel_spmd

    n_cores = 8
    B, Tm, X, Nn = x.shape          # 32, 256, 64, 256
    assert Nn == N
    rows = B * Tm * X
    rows_per_core = rows // n_cores
    s = float(np.asarray(scale))

    nc = _build(rows_per_core, s, **(build_kw or {}))
    xs = np.ascontiguousarray(np.asarray(x, dtype=np.float32)).reshape(
        n_cores, rows_per_core * N)
    in_maps = [{"x": xs[i]} for i in range(n_cores)]
    res = run_bass_kernel_spmd(nc, in_maps, list(range(n_cores)),
                               trace=trace, **kw)
    # partition p of core i holds rows [p*T, (p+1)*T) -> flat concat
    out = np.concatenate([r["out"].reshape(-1) for r in res.results], axis=0)
    return out.reshape(B, Tm, X).astype(np.float32), res


def kernel(x: np.ndarray, scale: np.ndarray) -> np.ndarray:
    return _run(x, scale)[0]
